# revision 22
# baseline (speedup 1.0000x reference)
"""BitLinear fake-quant GEMM on 8 trn2 NeuronCores, data-parallel over batch.

Per core: y[s,o] = round(clip(x/a_scale*127)) @ clip(round(w/w_scale),-1,1)^T
          * (w_scale * a_scale / 127),  a_scale = rowmax|x| + eps.

Quantized activations are integers |v|<=127 and weights are ternary, so a
bf16 matmul with fp32 PSUM accumulation is exact integer arithmetic.
"""

import os
import sys

import numpy as np

sys.path.insert(0, "/opt/trn_rl_repo")

import concourse.bacc as bacc
import concourse.mybir as mybir
import concourse.tile as tile
from concourse.bass_utils import run_bass_kernel_spmd

F32 = mybir.dt.float32
BF16 = mybir.dt.bfloat16
AF = mybir.ActivationFunctionType
ALU = mybir.AluOpType

B = 8      # batches == cores
S = 4096   # rows per core
D = 1024   # in features (contraction)
O = 1024   # out features
P = 128
GA = 4     # s-tiles per DMA group
KB = D // P
RND = 12582912.0  # 1.5*2**23: (z+RND)-RND == round-half-even(z) for |z|<2**22
EPS = 1e-8

_CACHE = {}
TRACE_DIR = None


def _build(s_rows=S):
    nt = s_rows // P
    ng = nt // GA
    nc = bacc.Bacc("TRN2", target_bir_lowering=False, debug=False)
    x_d = nc.dram_tensor("x", [s_rows, D], F32, kind="ExternalInput")
    w_d = nc.dram_tensor("weight", [O, D], F32, kind="ExternalInput")
    wsc_d = nc.dram_tensor("wsc", [1, 2], F32, kind="ExternalInput")
    y_d = nc.dram_tensor("y", [s_rows, O], F32, kind="ExternalOutput")
    xa, wa, sca, ya = x_d.ap(), w_d.ap(), wsc_d.ap(), y_d.ap()

    with tile.TileContext(nc) as tc:
        with (
            tc.tile_pool(name="wraw", bufs=1) as wraw_p,
            tc.tile_pool(name="wq", bufs=2) as wq_p,
            tc.tile_pool(name="wqT", bufs=1) as wqT_p,
            tc.tile_pool(name="xg", bufs=8) as xg_p,
            tc.tile_pool(name="stat", bufs=10) as stat_p,
            tc.tile_pool(name="quant", bufs=4) as q_p,
            tc.tile_pool(name="aqT", bufs=4) as aqT_p,
            tc.tile_pool(name="yout", bufs=3) as y_p,
            tc.tile_pool(name="psum", bufs=4, space="PSUM") as ps_p,
        ):
            # scalar broadcast: wsc = [1/w_scale, w_scale/127] -> all partitions
            wsc0 = wraw_p.tile([P, 2], F32, tag="wsc0")
            nc.sync.dma_start(out=wsc0[0:1, :], in_=sca[:, :])
            wscb = wraw_p.tile([P, 2], F32, tag="wscb")
            nc.gpsimd.partition_broadcast(wscb[:, :], wsc0[0:1, :], channels=P)
            recw_b = wscb[:, 0:1]
            ws127_b = wscb[:, 1:2]

            # weight: quantize to ternary bf16, then transpose to [i, o]
            wa3 = wa.rearrange("(a p) d -> p a d", p=P)
            wqT = wqT_p.tile([P, KB, O], BF16)  # [i-in-blk, i-blk, o]
            for k in range(KB):
                w_sb = wq_p.tile([P, D], F32, tag="wraw")
                nc.sync.dma_start(out=w_sb[:], in_=wa3[:, k, :])
                tw = wq_p.tile([P, D], F32, tag="tw")
                nc.scalar.activation(tw[:], w_sb[:], AF.Copy, bias=RND, scale=recw_b)
                tw2 = wq_p.tile([P, D], F32, tag="tw2")
                nc.vector.tensor_scalar(tw2[:], tw[:], RND, 1.0, ALU.subtract, ALU.min)
                wq = wq_p.tile([P, D], BF16, tag="wq")
                nc.vector.tensor_scalar(wq[:], tw2[:], -1.0, None, ALU.max)
                # batched xbar transpose: [128 o, 1024 i] -> i split over
                # (blk, part) in one instruction; exact i<->(blk,part) mapping
                # only needs to match the activation transpose below.
                nc.sync.dma_start_transpose(wqT[:, :, k * P:(k + 1) * P], wq[:])

            # DMA queue split: x loads on the ACT HWDGE queue, y stores on the
            # SWDGE (gpsimd) queue, transposes + weights on the SP HWDGE queue
            # (xbar transposes must stay on a single queue: shared-xbar hazard).
            #
            # Engine instruction streams are strictly in-order: one op waiting
            # on a semaphore blocks every later op on that engine. So stages
            # are emitted with explicit lookahead lags — loads far ahead,
            # stats ahead of quantize, epilogue lagged behind the matmuls —
            # to keep every stream's head dependency already satisfied.
            LOAD_LA = 6   # load for tile t+6 issued at slot t
            STAT_LA = 3   # stats chain for t+3 at slot t
            EPI_LAG = 3   # epilogue+store for t-3 at slot t
            xts, stats, quants, psums = {}, {}, {}, {}

            def emit_load(t):
                if not (0 <= t < nt):
                    return
                xt = xg_p.tile([P, D], F32, tag="xt")
                nc.gpsimd.dma_start(out=xt[:], in_=xa[t * P:(t + 1) * P, :])
                xts[t] = xt

            def emit_stats(t):
                if not (0 <= t < nt):
                    return
                xt = xts[t]
                st = stat_p.tile([P, 1], F32, tag="st")
                nc.vector.tensor_reduce(
                    st[:], xt[:], mybir.AxisListType.X, ALU.max,
                    apply_absolute_value=True,
                )
                ga_t = stat_p.tile([P, 1], F32, tag="ga")
                nc.vector.tensor_scalar(ga_t[:], st[:], EPS, None, ALU.add)
                rec127 = stat_p.tile([P, 1], F32, tag="rec127")
                nc.vector.reciprocal(rec127[:], ga_t[:])
                nc.vector.tensor_scalar(rec127[:], rec127[:], 127.0, None, ALU.mult)
                epi = stat_p.tile([P, 1], F32, tag="epi")
                nc.vector.tensor_scalar(epi[:], ga_t[:], ws127_b, None, ALU.mult)
                stats[t] = (rec127, epi)

            def emit_quant(t):
                if not (0 <= t < nt):
                    return
                xt = xts.pop(t)
                rec127, _ = stats[t]
                if t % 2 == 0:
                    quants["aq2"] = q_p.tile([P, 2, D], BF16, tag="aq", name="aq2")
                aq2 = quants["aq2"]
                tq = q_p.tile([P, D], F32, tag="tq")
                nc.vector.tensor_scalar(tq[:], xt[:], rec127[:], RND, ALU.mult, ALU.add)
                nc.vector.tensor_scalar(aq2[:, t % 2, :], tq[:], RND, None, ALU.subtract)
                if t % 2 == 1:
                    aqT = aqT_p.tile([P, 2 * KB, P], BF16)
                    nc.sync.dma_start_transpose(
                        aqT[:], aq2.rearrange("p a d -> p (a d)")
                    )
                    for half in range(2):
                        tt = t - 1 + half
                        yt = ps_p.tile([P, O], F32)
                        for b2 in range(KB):
                            blk = half * KB + b2
                            nc.tensor.matmul(
                                yt[:, 0:512], aqT[:, blk, :], wqT[:, b2, 0:512],
                                start=(b2 == 0), stop=(b2 == KB - 1),
                            )
                            nc.tensor.matmul(
                                yt[:, 512:1024], aqT[:, blk, :], wqT[:, b2, 512:1024],
                                start=(b2 == 0), stop=(b2 == KB - 1),
                            )
                        psums[tt] = yt

            def emit_epi(t):
                if not (0 <= t < nt):
                    return
                yt = psums.pop(t)
                _, epi = stats.pop(t)
                ysb = y_p.tile([P, O], F32)
                nc.scalar.activation(ysb[:], yt[:], AF.Copy, bias=0.0, scale=epi[:])
                nc.scalar.dma_start(out=ya[t * P:(t + 1) * P, :], in_=ysb[:])

            for t in range(min(LOAD_LA, nt)):
                emit_load(t)
            for t in range(min(STAT_LA, nt)):
                emit_stats(t)
            for slot in range(nt + EPI_LAG):
                emit_load(slot + LOAD_LA)
                emit_stats(slot + STAT_LA)
                emit_quant(slot)
                emit_epi(slot - EPI_LAG)
    nc.compile()
    return nc


def _scales(weight):
    # w_scale in fp64 then rounded, mirroring fp32 `mean(|w|) + eps` as closely
    # as any fp32 summation order allows.
    m = np.abs(weight.astype(np.float64)).mean()
    ws = np.float32(np.float32(m) + np.float32(EPS))
    recw = np.float32(1.0 / np.float64(ws))
    ws127 = np.float32(np.float64(ws) / 127.0)
    return np.array([[recw, ws127]], dtype=np.float32)


def kernel(x, weight):
    x = np.ascontiguousarray(np.asarray(x), dtype=np.float32)
    weight = np.ascontiguousarray(np.asarray(weight), dtype=np.float32)
    assert x.shape == (B, S, D) and weight.shape == (O, D)
    nc = _CACHE.get("nc")
    if nc is None:
        nc = _CACHE["nc"] = _build()
    wsc = _scales(weight)
    in_maps = [{"x": x[c], "weight": weight, "wsc": wsc} for c in range(B)]
    trace = bool(int(os.environ.get("BITLINEAR_TRACE", "0")))
    res = run_bass_kernel_spmd(
        nc, in_maps, list(range(B)), trace=trace, tmpdir=TRACE_DIR
    )
    _CACHE["last"] = res
    return np.stack([res.results[c]["y"] for c in range(B)], axis=0)


# revision 27
# speedup vs baseline: 1.0078x; 1.0078x over previous
"""BitLinear fake-quant GEMM on 8 trn2 NeuronCores, data-parallel over batch.

Per core: y[s,o] = round(clip(x/a_scale*127)) @ clip(round(w/w_scale),-1,1)^T
          * (w_scale * a_scale / 127),  a_scale = rowmax|x| + eps.

Quantized activations are integers |v|<=127 and weights are ternary, so a
bf16 matmul with fp32 PSUM accumulation is exact integer arithmetic.
"""

import os
import sys

import numpy as np

sys.path.insert(0, "/opt/trn_rl_repo")

import concourse.bacc as bacc
import concourse.mybir as mybir
import concourse.tile as tile
from concourse.bass_utils import run_bass_kernel_spmd

F32 = mybir.dt.float32
BF16 = mybir.dt.bfloat16
AF = mybir.ActivationFunctionType
ALU = mybir.AluOpType

B = 8      # batches == cores
S = 4096   # rows per core
D = 1024   # in features (contraction)
O = 1024   # out features
P = 128
GA = 4     # s-tiles per DMA group
KB = D // P
RND = 12582912.0  # 1.5*2**23: (z+RND)-RND == round-half-even(z) for |z|<2**22
EPS = 1e-8

_CACHE = {}
TRACE_DIR = None


def _build(s_rows=S):
    nt = s_rows // P
    ng = nt // GA
    nc = bacc.Bacc("TRN2", target_bir_lowering=False, debug=False)
    x_d = nc.dram_tensor("x", [s_rows, D], F32, kind="ExternalInput")
    w_d = nc.dram_tensor("weight", [O, D], F32, kind="ExternalInput")
    wsc_d = nc.dram_tensor("wsc", [P, 2], F32, kind="ExternalInput")
    y_d = nc.dram_tensor("y", [s_rows, O], F32, kind="ExternalOutput")
    xa, wa, sca, ya = x_d.ap(), w_d.ap(), wsc_d.ap(), y_d.ap()

    with tile.TileContext(nc) as tc:
        with (
            tc.tile_pool(name="wraw", bufs=1) as wraw_p,
            tc.tile_pool(name="wq", bufs=2) as wq_p,
            tc.tile_pool(name="wqT", bufs=1) as wqT_p,
            tc.tile_pool(name="xg", bufs=8) as xg_p,
            tc.tile_pool(name="stat", bufs=10) as stat_p,
            tc.tile_pool(name="quant", bufs=4) as q_p,
            tc.tile_pool(name="aqT", bufs=4) as aqT_p,
            tc.tile_pool(name="yout", bufs=3) as y_p,
            tc.tile_pool(name="psum", bufs=4, space="PSUM") as ps_p,
        ):
            # wsc = [1/w_scale, w_scale/127], pre-broadcast to 128 partitions
            # on the host so nothing gates on a partition_broadcast.
            wscb = wraw_p.tile([P, 2], F32, tag="wscb")
            nc.sync.dma_start(out=wscb[:], in_=sca[:, :])
            recw_b = wscb[:, 0:1]
            ws127_b = wscb[:, 1:2]

            # weight: quantize to ternary bf16, then transpose to [i, o].
            # All loads first: a transpose ahead of a load on the in-order SP
            # stream would stall every later load behind its input deps.
            wa3 = wa.rearrange("(a p) d -> p a d", p=P)
            wqT = wqT_p.tile([P, KB, O], BF16)  # [i-in-blk, i-blk, o]
            w_sbs, wqs = [], []
            for k in range(KB):
                w_sb = wq_p.tile([P, D], F32, tag=f"wraw{k}", name=f"wraw{k}", bufs=1)
                nc.sync.dma_start(out=w_sb[:], in_=wa3[:, k, :])
                w_sbs.append(w_sb)
            for k in range(KB):
                tw = wq_p.tile([P, D], F32, tag="tw")
                nc.scalar.activation(tw[:], w_sbs[k][:], AF.Copy, bias=RND, scale=recw_b)
                tw2 = wq_p.tile([P, D], F32, tag="tw2")
                nc.vector.tensor_scalar(tw2[:], tw[:], RND, 1.0, ALU.subtract, ALU.min)
                wq = wq_p.tile([P, D], BF16, tag=f"wq{k}", name=f"wq{k}", bufs=1)
                nc.vector.tensor_scalar(wq[:], tw2[:], -1.0, None, ALU.max)
                wqs.append(wq)
            for k in range(KB):
                # batched xbar transpose: [128 o, 1024 i] -> i split over
                # (blk, part) in one instruction; exact i<->(blk,part) mapping
                # only needs to match the activation transpose below.
                nc.sync.dma_start_transpose(wqT[:, :, k * P:(k + 1) * P], wqs[k][:])

            # DMA queue split: x loads on the ACT HWDGE queue, y stores on the
            # SWDGE (gpsimd) queue, transposes + weights on the SP HWDGE queue
            # (xbar transposes must stay on a single queue: shared-xbar hazard).
            #
            # Engine instruction streams are strictly in-order: one op waiting
            # on a semaphore blocks every later op on that engine. So stages
            # are emitted with explicit lookahead lags — loads far ahead,
            # stats ahead of quantize, epilogue lagged behind the matmuls —
            # to keep every stream's head dependency already satisfied.
            LOAD_LA = 6   # load for tile t+6 issued at slot t
            STAT_LA = 3   # stats chain for t+3 at slot t
            EPI_LAG = 3   # epilogue+store for t-3 at slot t
            xts, stats, quants, psums = {}, {}, {}, {}

            def emit_load(t):
                if not (0 <= t < nt):
                    return
                xt = xg_p.tile([P, D], F32, tag="xt")
                nc.gpsimd.dma_start(out=xt[:], in_=xa[t * P:(t + 1) * P, :])
                xts[t] = xt

            def emit_stats(t):
                if not (0 <= t < nt):
                    return
                xt = xts[t]
                st = stat_p.tile([P, 1], F32, tag="st")
                nc.vector.tensor_reduce(
                    st[:], xt[:], mybir.AxisListType.X, ALU.max,
                    apply_absolute_value=True,
                )
                ga_t = stat_p.tile([P, 1], F32, tag="ga")
                nc.vector.tensor_scalar(ga_t[:], st[:], EPS, None, ALU.add)
                rec127 = stat_p.tile([P, 1], F32, tag="rec127")
                nc.vector.reciprocal(rec127[:], ga_t[:])
                nc.vector.tensor_scalar(rec127[:], rec127[:], 127.0, None, ALU.mult)
                epi = stat_p.tile([P, 1], F32, tag="epi")
                nc.vector.tensor_scalar(epi[:], ga_t[:], ws127_b, None, ALU.mult)
                stats[t] = (rec127, epi)

            def emit_quant(t):
                if not (0 <= t < nt):
                    return
                xt = xts.pop(t)
                rec127, _ = stats[t]
                if t % 2 == 0:
                    quants["aq2"] = q_p.tile([P, 2, D], BF16, tag="aq", name="aq2")
                aq2 = quants["aq2"]
                tq = q_p.tile([P, D], F32, tag="tq")
                nc.vector.tensor_scalar(tq[:], xt[:], rec127[:], RND, ALU.mult, ALU.add)
                nc.vector.tensor_scalar(aq2[:, t % 2, :], tq[:], RND, None, ALU.subtract)
                if t % 2 == 1:
                    aqT = aqT_p.tile([P, 2 * KB, P], BF16)
                    nc.sync.dma_start_transpose(
                        aqT[:], aq2.rearrange("p a d -> p (a d)")
                    )
                    for half in range(2):
                        tt = t - 1 + half
                        yt = ps_p.tile([P, O], F32)
                        for b2 in range(KB):
                            blk = half * KB + b2
                            nc.tensor.matmul(
                                yt[:, 0:512], aqT[:, blk, :], wqT[:, b2, 0:512],
                                start=(b2 == 0), stop=(b2 == KB - 1),
                            )
                            nc.tensor.matmul(
                                yt[:, 512:1024], aqT[:, blk, :], wqT[:, b2, 512:1024],
                                start=(b2 == 0), stop=(b2 == KB - 1),
                            )
                        psums[tt] = yt

            def emit_epi(t):
                if not (0 <= t < nt):
                    return
                yt = psums.pop(t)
                _, epi = stats.pop(t)
                ysb = y_p.tile([P, O], F32)
                nc.scalar.activation(ysb[:], yt[:], AF.Copy, bias=0.0, scale=epi[:])
                nc.scalar.dma_start(out=ya[t * P:(t + 1) * P, :], in_=ysb[:])

            for t in range(min(LOAD_LA, nt)):
                emit_load(t)
            for t in range(min(STAT_LA, nt)):
                emit_stats(t)
            for slot in range(nt + EPI_LAG):
                emit_load(slot + LOAD_LA)
                emit_stats(slot + STAT_LA)
                emit_quant(slot)
                emit_epi(slot - EPI_LAG)
    nc.compile()
    return nc


def _scales(weight):
    # w_scale in fp64 then rounded, mirroring fp32 `mean(|w|) + eps` as closely
    # as any fp32 summation order allows.
    m = np.abs(weight.astype(np.float64)).mean()
    ws = np.float32(np.float32(m) + np.float32(EPS))
    recw = np.float32(1.0 / np.float64(ws))
    ws127 = np.float32(np.float64(ws) / 127.0)
    return np.array([[recw, ws127]], dtype=np.float32)


def kernel(x, weight):
    x = np.ascontiguousarray(np.asarray(x), dtype=np.float32)
    weight = np.ascontiguousarray(np.asarray(weight), dtype=np.float32)
    assert x.shape == (B, S, D) and weight.shape == (O, D)
    nc = _CACHE.get("nc")
    if nc is None:
        nc = _CACHE["nc"] = _build()
    wsc = np.tile(_scales(weight), (P, 1))
    in_maps = [{"x": x[c], "weight": weight, "wsc": wsc} for c in range(B)]
    trace = bool(int(os.environ.get("BITLINEAR_TRACE", "0")))
    res = run_bass_kernel_spmd(
        nc, in_maps, list(range(B)), trace=trace, tmpdir=TRACE_DIR
    )
    _CACHE["last"] = res
    return np.stack([res.results[c]["y"] for c in range(B)], axis=0)
